# revision 27
# baseline (speedup 1.0000x reference)
"""Trainium2 Bass kernel for nn_Attention_41996190220419.

Single-head causal attention with softplus weights and a time-flipped
rotary embedding, B=8 T=2048 C=1024 fp32.

Sharding: pure data-parallel over batch (1 batch element per NeuronCore,
8 cores, no collectives).

Per-core kernel (matmuls bf16 with fp32 PSUM accumulation, except the
score matmul which runs fp8e4 in DoubleRow mode at 2 MACs/cell/cycle):
  phase 1: xT [C,T] arrives pre-transposed from the host.
           V = x Wv + bv into resident SBUF tiles ([T,C] layout, 1024-wide
           moving operands), then KT/QT = W^T x^T (+bias via ACT), rotary
           rotation on DVE (cos/sin streamed, scaled x16) writing fp8e4
           pair-tiles [128, 2, T] (pair = channel groups e and e+4).
  phase 2: per 512-wide i-span: ST[j,i] = sum_c QR[j,c] KR[i,c] for the
           causal (block lower-triangle) blocks only via 4 DoubleRow
           matmuls, then softplus = Ln(Exp(x/(256 sqrt(C)))+1) on ACT,
           diagonal-block 0/1 masks on DVE
  phase 3: OT[c,i] = sum_j V[j,c] ST[j,i]  (PSUM accumulate over j, bf16)
  phase 4: OUT[t,cout] = sum_c OT[c,t] Wp[c,cout] + bp -> DRAM

The even/odd rotation pairs are turned into tile-level structure by
permuting the columns of Wk/Wq (and bk/bq) on the host to [evens|odds];
scores are invariant to any channel permutation applied to both K and Q.
cos/sin tables (x16, folded into the fp8 score scale) and diagonal masks
are precomputed on the host and passed as extra inputs.
"""

import os
import sys

if "/opt/trn_rl_repo" not in sys.path:
    sys.path.insert(0, "/opt/trn_rl_repo")

import numpy as np
import ml_dtypes

import concourse.bass as bass
import concourse.bacc as bacc
import concourse.mybir as mybir
import concourse.tile as tile
from concourse.bass_utils import run_bass_kernel_spmd

B, T, C = 8, 2048, 1024
H = C // 2
NCORES = 8
PD = 128
TCH = 512                 # i-span width (attention)
WCH = 1024                # moving-operand width for the dense GEMMs
NT = T // PD              # 16
NSP = T // TCH            # 4
NW = T // WCH             # 2
NG = C // PD              # 8
NP = NG // 2              # 4 fp8 pair-tiles
BF16 = mybir.dt.bfloat16
F32 = mybir.dt.float32
FP8 = mybir.dt.float8e4
AF = mybir.ActivationFunctionType
DR = mybir.MatmulPerfMode.DoubleRow
RSCALE = 16.0             # fp8 rotation output scale (folded into trig)
SCORE_SCALE = float(C) ** -0.5 / (RSCALE * RSCALE)
X8SCALE = 16.0            # fp8 x scale for the K/Q GEMMs
W8SCALE = 4096.0          # fp8 Wk/Wq scale
KQ_SCALE = 1.0 / (X8SCALE * W8SCALE)

_CACHE = {}

LAST_RESULT = None  # BassKernelResults of the most recent run (for profiling)


def _patch_act_tables():
    """Force every ACT func we use (Copy/Identity/Exp/Ln) to resolve to the
    single `natural_log_exp_and_others` table so the Exp/Ln alternation in
    the softplus does not thrash ACT_TABLE_LOADs (1.3us each).
    Table ids are positional, so keep the dict order and only strip
    functions from the other tables."""
    if _CACHE.get("act_patched"):
        return
    from concourse import hw_specs
    orig = hw_specs.get_activation_tables
    combined = "natural_log_exp_and_others"

    def patched(arch):
        tables = orig(arch)
        if combined in tables:
            keep = tables[combined]
            tables = {
                name: (s if name == combined else (s - keep))
                for name, s in tables.items()
            }
        return tables

    hw_specs.get_activation_tables = patched
    bacc.get_activation_tables = patched
    _CACHE["act_patched"] = True


def _build_nc():
    _patch_act_tables()
    nc = bacc.Bacc("TRN2", target_bir_lowering=False, debug=False,
                   num_devices=NCORES)

    xt_d = nc.dram_tensor("xt", [C, T], BF16, kind="ExternalInput").ap()
    xt8_d = nc.dram_tensor("xt8", [C // 2, 2 * T], FP8, kind="ExternalInput").ap()
    wk8_d = nc.dram_tensor("wk8", [C // 2, 2 * C], FP8, kind="ExternalInput").ap()
    wq8_d = nc.dram_tensor("wq8", [C // 2, 2 * C], FP8, kind="ExternalInput").ap()
    wv_d = nc.dram_tensor("wv", [C, C], BF16, kind="ExternalInput").ap()
    wp_d = nc.dram_tensor("wp", [C, C], BF16, kind="ExternalInput").ap()
    bkr_d = nc.dram_tensor("bkr", [PD, NG], F32, kind="ExternalInput").ap()
    bqr_d = nc.dram_tensor("bqr", [PD, NG], F32, kind="ExternalInput").ap()
    bvb_d = nc.dram_tensor("bvb", [PD, C], F32, kind="ExternalInput").ap()
    bpb_d = nc.dram_tensor("bpb", [PD, C], F32, kind="ExternalInput").ap()
    cos_d = nc.dram_tensor("cosT", [H, T], BF16, kind="ExternalInput").ap()
    sin_d = nc.dram_tensor("sinT", [H, T], BF16, kind="ExternalInput").ap()
    msk_d = nc.dram_tensor("masks", [NSP, PD, TCH], BF16,
                           kind="ExternalInput").ap()
    out_d = nc.dram_tensor("out", [T, C], F32, kind="ExternalOutput").ap()

    with tile.TileContext(nc) as tc:
        with tc.tile_pool(name="persist", bufs=1) as pp:

            # fp8 pair-tiles: dim1 index 0 -> channel group e, 1 -> e+4
            krt = [pp.tile([PD, 2, T], FP8, tag=f"krt{e}", name=f"krt{e}")
                   for e in range(NP)]
            qrt = [pp.tile([PD, 2, T], FP8, tag=f"qrt{e}", name=f"qrt{e}")
                   for e in range(NP)]
            vsb = [pp.tile([PD, C], BF16, tag=f"v{j}", name=f"v{j}")
                   for j in range(NT)]

            bkr = pp.tile([PD, NG], F32, name="bkr")
            nc.sync.dma_start(out=bkr, in_=bkr_d)
            bqr = pp.tile([PD, NG], F32, name="bqr")
            nc.sync.dma_start(out=bqr, in_=bqr_d)
            bvb = pp.tile([PD, C], F32, name="bvb")
            nc.sync.dma_start(out=bvb, in_=bvb_d)
            bpb = pp.tile([PD, C], F32, name="bpb")
            nc.sync.dma_start(out=bpb, in_=bpb_d)
            mskt = []
            for d in range(NSP):
                m = pp.tile([PD, TCH], BF16, tag=f"msk{d}", name=f"msk{d}")
                nc.sync.dma_start(out=m, in_=msk_d[d])
                mskt.append(m)

            # ---------------- phase 1: K/Q (rotated), then V --------------
            # K/Q first: they need only ~4MB of fp8 DMA before compute can
            # start, and running V afterwards hides the rotation DVE chain
            # under the V GEMMs (DVE queue order: K-rot, Q-rot, V-bias).
            with tc.tile_pool(name="p1", bufs=1) as p1, \
                 tc.tile_pool(name="ps1", bufs=6, space="PSUM") as ps1:
                xt8 = [p1.tile([PD, 2, T], FP8, tag=f"xt8_{c2}",
                               name=f"xt8_{c2}") for c2 in range(NP)]
                for c2 in range(NP):
                    nc.sync.dma_start(out=xt8[c2],
                                      in_=xt8_d[c2 * PD:(c2 + 1) * PD, :])
                xt = [p1.tile([PD, T], BF16, tag=f"xt{g}", name=f"xt{g}")
                      for g in range(NG)]

                # K and Q GEMMs (fp8 DoubleRow) + rotation (cos/sin streamed)
                for wname, w_d, brt, dst in (("k", wk8_d, bkr, krt),
                                             ("q", wq8_d, bqr, qrt)):
                    wsb = []
                    for c2 in range(NP):
                        wt = p1.tile([PD, 2, C], FP8, tag="w8", bufs=8,
                                     name=f"w{wname}{c2}")
                        nc.sync.dma_start(out=wt, in_=w_d[c2 * PD:(c2 + 1) * PD, :])
                        wsb.append(wt)
                    for e in range(NP):
                        o = e + NP
                        trig = {}
                        for ch in range(NSP):
                            csl = slice(ch * TCH, (ch + 1) * TCH)
                            cs = p1.tile([PD, TCH], BF16, tag="trig", bufs=12,
                                         name=f"cs{wname}{e}_{ch}")
                            nc.sync.dma_start(
                                out=cs, in_=cos_d[e * PD:(e + 1) * PD, csl])
                            sn = p1.tile([PD, TCH], BF16, tag="trig", bufs=12,
                                         name=f"sn{wname}{e}_{ch}")
                            nc.sync.dma_start(
                                out=sn, in_=sin_d[e * PD:(e + 1) * PD, csl])
                            trig[ch] = (cs, sn)
                        for ch in range(NSP):
                            tmp = {}
                            for g in (e, o):
                                ps = ps1.tile([PD, TCH], F32, tag="ps_mm",
                                              name=f"pkq{wname}{g}_{ch}")
                                for c2 in range(NP):
                                    nc.tensor.matmul(
                                        ps,
                                        lhsT=wsb[c2][:, :, g * PD:(g + 1) * PD],
                                        rhs=xt8[c2][:, :, ch * TCH:(ch + 1) * TCH],
                                        perf_mode=DR,
                                        start=(c2 == 0), stop=(c2 == NP - 1))
                                kt = p1.tile([PD, TCH], BF16, tag="kttmp",
                                             bufs=10, name=f"kt{wname}{g}_{ch}")
                                nc.scalar.activation(kt, ps, AF.Identity,
                                                     bias=brt[:, g:g + 1],
                                                     scale=KQ_SCALE)
                                tmp[g] = kt
                            sl = slice(ch * TCH, (ch + 1) * TCH)
                            cs, sn = trig[ch]
                            ze, zo = tmp[e], tmp[o]
                            t1 = p1.tile([PD, TCH], BF16, tag="rot", bufs=6,
                                         name=f"r1{wname}{e}_{ch}")
                            nc.vector.tensor_mul(t1, ze, cs)
                            t2 = p1.tile([PD, TCH], BF16, tag="rot", bufs=6,
                                         name=f"r2{wname}{e}_{ch}")
                            nc.vector.tensor_mul(t2, zo, sn)
                            nc.vector.tensor_add(dst[e][:, 0, sl], t1, t2)
                            t3 = p1.tile([PD, TCH], BF16, tag="rot", bufs=6,
                                         name=f"r3{wname}{e}_{ch}")
                            nc.vector.tensor_mul(t3, zo, cs)
                            t4 = p1.tile([PD, TCH], BF16, tag="rot", bufs=6,
                                         name=f"r4{wname}{e}_{ch}")
                            nc.vector.tensor_mul(t4, ze, sn)
                            nc.vector.tensor_sub(dst[e][:, 1, sl], t3, t4)

                # V GEMM last: overlaps the PE with the K/Q rotation DVE
                # chain; writes straight into resident vsb tiles so the OT
                # matmuls never wait on a DRAM round-trip
                for g in range(NG):
                    nc.sync.dma_start(out=xt[g],
                                      in_=xt_d[g * PD:(g + 1) * PD, :])
                wsb = []
                for ci in range(NG):
                    wt = p1.tile([PD, C], BF16, tag="w", bufs=8, name=f"wv{ci}")
                    nc.sync.dma_start(out=wt, in_=wv_d[ci * PD:(ci + 1) * PD, :])
                    wsb.append(wt)
                for tt in range(NT):
                    for h in range(2):
                        ps = ps1.tile([PD, TCH], F32, tag="ps_mm",
                                      name=f"pv{tt}_{h}")
                        for ci in range(NG):
                            nc.tensor.matmul(
                                ps,
                                lhsT=xt[ci][:, tt * PD:(tt + 1) * PD],
                                rhs=wsb[ci][:, h * TCH:(h + 1) * TCH],
                                start=(ci == 0), stop=(ci == NG - 1))
                        # ACT drains the psum (fp32, no extra rounding);
                        # the idle GpSimd engine then adds the bias, keeping
                        # the V path off the rotation-busy DVE entirely
                        vt = p1.tile([PD, TCH], F32, tag="vtmp", bufs=4,
                                     name=f"vt{tt}_{h}")
                        nc.scalar.activation(vt, ps, AF.Copy)
                        nc.gpsimd.tensor_add(vsb[tt][:, h * TCH:(h + 1) * TCH],
                                             vt, bvb[:, h * TCH:(h + 1) * TCH])

            # ---------------- phases 2-4: attention + projection ---------
            with tc.tile_pool(name="at", bufs=1) as at, \
                 tc.tile_pool(name="psA", bufs=4, space="PSUM") as psA, \
                 tc.tile_pool(name="psB", bufs=2, space="PSUM") as psB, \
                 tc.tile_pool(name="psC", bufs=2, space="PSUM") as psC:
                wpsb = []
                for ci in range(NG):
                    wt = at.tile([PD, C], BF16, tag=f"wp{ci}", name=f"wp{ci}")
                    nc.sync.dma_start(out=wt, in_=wp_d[ci * PD:(ci + 1) * PD, :])
                    wpsb.append(wt)

                for s in range(NSP):
                    nj = 4 * (s + 1)
                    stact = []
                    for j in range(nj):
                        # diagonal blocks (d = 0..3): columns < 128*d of the
                        # 512-wide i-span are fully below the causal mask —
                        # compute only the right 512-128*d columns
                        d = j - 4 * s
                        off = PD * d if d > 0 else 0
                        isl = slice(s * TCH + off, (s + 1) * TCH)
                        ps = psA.tile([PD, TCH], F32, tag="ps_mm",
                                      name=f"pst{s}_{j}")
                        for e in range(NP):
                            nc.tensor.matmul(
                                ps[:, off:],
                                lhsT=qrt[e][:, :, j * PD:(j + 1) * PD],
                                rhs=krt[e][:, :, isl],
                                perf_mode=DR,
                                start=(e == 0), stop=(e == NP - 1))
                        # softplus(x) = ln(1 + exp(x)); scores/sqrt(C) are
                        # bounded to a few units so exp cannot overflow
                        se = at.tile([PD, TCH], F32, tag="stexp", bufs=4,
                                     name=f"se{s}_{j}")
                        nc.scalar.activation(se[:, off:], ps[:, off:],
                                             AF.Exp, scale=SCORE_SCALE)
                        st = at.tile([PD, TCH], BF16, tag="stact", bufs=20,
                                     name=f"st{s}_{j}")
                        nc.scalar.activation(st[:, off:], se[:, off:],
                                             AF.Ln, bias=1.0)
                        if d >= 0:
                            nc.vector.tensor_mul(st[:, off:], st[:, off:],
                                                 mskt[d][:, off:])
                        stact.append((st, off))

                    ot = []
                    for g in range(NG):
                        ps2 = psB.tile([PD, TCH], F32, tag="ps_ot",
                                       name=f"pot{s}_{g}")
                        for j in range(nj):
                            st, off = stact[j]
                            nc.tensor.matmul(
                                ps2[:, off:],
                                lhsT=vsb[j][:, g * PD:(g + 1) * PD],
                                rhs=st[:, off:],
                                start=(j == 0), stop=(j == nj - 1))
                        o = at.tile([PD, TCH], BF16, tag="ot", bufs=16,
                                    name=f"ot{s}_{g}")
                        nc.scalar.activation(o, ps2, AF.Copy)
                        ot.append(o)

                    for tt in range(4):
                        trow = s * TCH + tt * PD
                        for h in range(2):
                            ps = psC.tile([PD, TCH], F32, tag="ps_pr",
                                          name=f"ppr{s}_{tt}_{h}")
                            for g in range(NG):
                                nc.tensor.matmul(
                                    ps,
                                    lhsT=ot[g][:, tt * PD:(tt + 1) * PD],
                                    rhs=wpsb[g][:, h * TCH:(h + 1) * TCH],
                                    start=(g == 0), stop=(g == NG - 1))
                            ob = at.tile([PD, TCH], F32, tag="ob", bufs=4,
                                         name=f"ob{s}_{tt}_{h}")
                            nc.vector.tensor_add(ob, ps,
                                                 bpb[:, h * TCH:(h + 1) * TCH])
                            nc.sync.dma_start(
                                out=out_d[trow:trow + PD, h * TCH:(h + 1) * TCH],
                                in_=ob)
    nc.finalize()
    return nc


def _static_tables():
    if "tables" in _CACHE:
        return _CACHE["tables"]
    perm = np.concatenate([np.arange(0, C, 2), np.arange(1, C, 2)])
    j = np.arange(H, dtype=np.float64)
    t = (T - 1 - np.arange(T)).astype(np.float64)
    ang = np.outer(j, t)                      # [H, T], angle of pair j at time t
    cosT = (np.cos(ang) * RSCALE).astype(ml_dtypes.bfloat16)
    sinT = (np.sin(ang) * RSCALE).astype(ml_dtypes.bfloat16)
    a = np.arange(PD)[:, None]
    b = np.arange(TCH)[None, :]
    masks = np.stack([(a + PD * d <= b) for d in range(NSP)])
    masks = masks.astype(ml_dtypes.bfloat16)
    _CACHE["tables"] = (perm, cosT, sinT, masks)
    return _CACHE["tables"]


def prepare(x, Wk, bk, Wq, bq, Wv, bv, Wp, bp):
    """Build (cached) the Bass program and the per-core input maps."""
    x = np.asarray(x, dtype=np.float32)
    Wk, bk = np.asarray(Wk, np.float32), np.asarray(bk, np.float32)
    Wq, bq = np.asarray(Wq, np.float32), np.asarray(bq, np.float32)
    Wv, bv = np.asarray(Wv, np.float32), np.asarray(bv, np.float32)
    Wp, bp = np.asarray(Wp, np.float32), np.asarray(bp, np.float32)

    perm, cosT, sinT, masks = _static_tables()

    def pair_fp8(arr, scale):
        """[C, F] -> [C/2, 2F] fp8: row pairs (2c2*128+p, (2c2+1)*128+p)
        interleaved along the free dim for DoubleRow contraction."""
        a = np.clip(arr * scale, -240, 240).astype(ml_dtypes.float8_e4m3fn)
        F = a.shape[1]
        a = a.reshape(NP, 2, PD, F).transpose(0, 2, 1, 3)
        return np.ascontiguousarray(a.reshape(C // 2, 2 * F))

    wk8 = pair_fp8(np.ascontiguousarray(Wk[:, perm]), W8SCALE)
    wq8 = pair_fp8(np.ascontiguousarray(Wq[:, perm]), W8SCALE)
    wv = Wv.astype(ml_dtypes.bfloat16)
    wp = Wp.astype(ml_dtypes.bfloat16)
    bkr = np.ascontiguousarray(bk[perm].reshape(NG, PD).T).astype(np.float32)
    bqr = np.ascontiguousarray(bq[perm].reshape(NG, PD).T).astype(np.float32)
    bvb = np.ascontiguousarray(np.broadcast_to(bv, (PD, C))).astype(np.float32)
    bpb = np.ascontiguousarray(np.broadcast_to(bp, (PD, C))).astype(np.float32)

    if "nc" not in _CACHE:
        _CACHE["nc"] = _build_nc()
    nc = _CACHE["nc"]

    shared = dict(wk8=wk8, wq8=wq8, wv=wv, wp=wp, bkr=bkr, bqr=bqr,
                  bvb=bvb, bpb=bpb, cosT=cosT, sinT=sinT, masks=masks)
    xb = x.astype(ml_dtypes.bfloat16)
    in_maps = []
    for i in range(NCORES):
        xti = np.ascontiguousarray(xb[i].T)
        xt8i = pair_fp8(xti.astype(np.float32), X8SCALE)
        in_maps.append(dict(xt=xti, xt8=xt8i, **shared))
    return nc, in_maps


def kernel(x, Wk, bk, Wq, bq, Wv, bv, Wp, bp):
    global LAST_RESULT
    nc, in_maps = prepare(x, Wk, bk, Wq, bq, Wv, bv, Wp, bp)
    res = run_bass_kernel_spmd(nc, in_maps, list(range(NCORES)))
    LAST_RESULT = res
    out = np.stack([res.results[i]["out"] for i in range(NCORES)], axis=0)
    return out.astype(np.float32)


# revision 28
# speedup vs baseline: 1.1942x; 1.1942x over previous
"""Trainium2 Bass kernel for nn_Attention_41996190220419.

Single-head causal attention with softplus weights and a time-flipped
rotary embedding, B=8 T=2048 C=1024 fp32.

Sharding: pure data-parallel over batch (1 batch element per NeuronCore,
8 cores, no collectives).

Per-core kernel (matmuls bf16 with fp32 PSUM accumulation, except the
score matmul which runs fp8e4 in DoubleRow mode at 2 MACs/cell/cycle):
  phase 1: xT [C,T] arrives pre-transposed from the host.
           V = x Wv + bv into resident SBUF tiles ([T,C] layout, 1024-wide
           moving operands), then KT/QT = W^T x^T (+bias via ACT), rotary
           rotation on DVE (cos/sin streamed, scaled x16) writing fp8e4
           pair-tiles [128, 2, T] (pair = channel groups e and e+4).
  phase 2: per 512-wide i-span: ST[j,i] = sum_c QR[j,c] KR[i,c] for the
           causal (block lower-triangle) blocks only via 4 DoubleRow
           matmuls, then softplus = Ln(Exp(x/(256 sqrt(C)))+1) on ACT,
           diagonal-block 0/1 masks on DVE
  phase 3: OT[c,i] = sum_j V[j,c] ST[j,i]  (PSUM accumulate over j, bf16)
  phase 4: OUT[t,cout] = sum_c OT[c,t] Wp[c,cout] + bp -> DRAM

The even/odd rotation pairs are turned into tile-level structure by
permuting the columns of Wk/Wq (and bk/bq) on the host to [evens|odds];
scores are invariant to any channel permutation applied to both K and Q.
cos/sin tables (x16, folded into the fp8 score scale) and diagonal masks
are precomputed on the host and passed as extra inputs.
"""

import os
import sys

if "/opt/trn_rl_repo" not in sys.path:
    sys.path.insert(0, "/opt/trn_rl_repo")

import numpy as np
import ml_dtypes

import concourse.bass as bass
import concourse.bacc as bacc
import concourse.mybir as mybir
import concourse.tile as tile
from concourse.bass_utils import run_bass_kernel_spmd

B, T, C = 8, 2048, 1024
H = C // 2
NCORES = 8
PD = 128
TCH = 512                 # i-span width (attention)
WCH = 1024                # moving-operand width for the dense GEMMs
NT = T // PD              # 16
NSP = T // TCH            # 4
NW = T // WCH             # 2
NG = C // PD              # 8
NP = NG // 2              # 4 fp8 pair-tiles
BF16 = mybir.dt.bfloat16
F32 = mybir.dt.float32
FP8 = mybir.dt.float8e4
AF = mybir.ActivationFunctionType
DR = mybir.MatmulPerfMode.DoubleRow
RSCALE = 16.0             # fp8 rotation output scale (folded into trig)
SCORE_SCALE = float(C) ** -0.5 / (RSCALE * RSCALE)
X8SCALE = 16.0            # fp8 x scale for the K/Q GEMMs
W8SCALE = 4096.0          # fp8 Wk/Wq scale
KQ_SCALE = 1.0 / (X8SCALE * W8SCALE)

_CACHE = {}

LAST_RESULT = None  # BassKernelResults of the most recent run (for profiling)


def _patch_act_tables():
    """Force every ACT func we use (Copy/Identity/Exp/Ln) to resolve to the
    single `natural_log_exp_and_others` table so the Exp/Ln alternation in
    the softplus does not thrash ACT_TABLE_LOADs (1.3us each).
    Table ids are positional, so keep the dict order and only strip
    functions from the other tables."""
    if _CACHE.get("act_patched"):
        return
    from concourse import hw_specs
    orig = hw_specs.get_activation_tables
    combined = "natural_log_exp_and_others"

    def patched(arch):
        tables = orig(arch)
        if combined in tables:
            keep = tables[combined]
            tables = {
                name: (s if name == combined else (s - keep))
                for name, s in tables.items()
            }
        return tables

    hw_specs.get_activation_tables = patched
    bacc.get_activation_tables = patched
    _CACHE["act_patched"] = True


def _build_nc():
    _patch_act_tables()
    nc = bacc.Bacc("TRN2", target_bir_lowering=False, debug=False,
                   num_devices=NCORES)

    xt_d = nc.dram_tensor("xt", [C, T], BF16, kind="ExternalInput").ap()
    xt8_d = nc.dram_tensor("xt8", [C // 2, 2 * T], FP8, kind="ExternalInput").ap()
    wk8_d = nc.dram_tensor("wk8", [C // 2, 2 * C], FP8, kind="ExternalInput").ap()
    wq8_d = nc.dram_tensor("wq8", [C // 2, 2 * C], FP8, kind="ExternalInput").ap()
    wv_d = nc.dram_tensor("wv", [C, C], BF16, kind="ExternalInput").ap()
    wp_d = nc.dram_tensor("wp", [C, C], BF16, kind="ExternalInput").ap()
    bkr_d = nc.dram_tensor("bkr", [PD, NG], F32, kind="ExternalInput").ap()
    bqr_d = nc.dram_tensor("bqr", [PD, NG], F32, kind="ExternalInput").ap()
    bvb_d = nc.dram_tensor("bvb", [PD, C], F32, kind="ExternalInput").ap()
    bpb_d = nc.dram_tensor("bpb", [PD, C], F32, kind="ExternalInput").ap()
    cos_d = nc.dram_tensor("cosT", [H, T], BF16, kind="ExternalInput").ap()
    sin_d = nc.dram_tensor("sinT", [H, T], BF16, kind="ExternalInput").ap()
    msk_d = nc.dram_tensor("masks", [NSP, PD, TCH], BF16,
                           kind="ExternalInput").ap()
    out_d = nc.dram_tensor("out", [T, C], F32, kind="ExternalOutput").ap()

    with tile.TileContext(nc) as tc:
        with tc.tile_pool(name="persist", bufs=1) as pp:

            # fp8 pair-tiles: dim1 index 0 -> channel group e, 1 -> e+4
            krt = [pp.tile([PD, 2, T], FP8, tag=f"krt{e}", name=f"krt{e}")
                   for e in range(NP)]
            qrt = [pp.tile([PD, 2, T], FP8, tag=f"qrt{e}", name=f"qrt{e}")
                   for e in range(NP)]
            vsb = [pp.tile([PD, C], BF16, tag=f"v{j}", name=f"v{j}")
                   for j in range(NT)]

            bkr = pp.tile([PD, NG], F32, name="bkr")
            nc.sync.dma_start(out=bkr, in_=bkr_d)
            bqr = pp.tile([PD, NG], F32, name="bqr")
            nc.sync.dma_start(out=bqr, in_=bqr_d)
            bvb = pp.tile([PD, C], F32, name="bvb")
            nc.sync.dma_start(out=bvb, in_=bvb_d)
            bpb = pp.tile([PD, C], F32, name="bpb")
            nc.sync.dma_start(out=bpb, in_=bpb_d)
            mskt = []
            for d in range(NSP):
                m = pp.tile([PD, TCH], BF16, tag=f"msk{d}", name=f"msk{d}")
                nc.sync.dma_start(out=m, in_=msk_d[d])
                mskt.append(m)

            # ---------------- phase 1: K/Q (rotated), then V --------------
            # K/Q first: they need only ~4MB of fp8 DMA before compute can
            # start, and running V afterwards hides the rotation DVE chain
            # under the V GEMMs (DVE queue order: K-rot, Q-rot, V-bias).
            with tc.tile_pool(name="p1", bufs=1) as p1, \
                 tc.tile_pool(name="ps1", bufs=6, space="PSUM") as ps1:
                xt8 = [p1.tile([PD, 2, T], FP8, tag=f"xt8_{c2}",
                               name=f"xt8_{c2}") for c2 in range(NP)]
                for c2 in range(NP):
                    nc.sync.dma_start(out=xt8[c2],
                                      in_=xt8_d[c2 * PD:(c2 + 1) * PD, :])
                xt = [p1.tile([PD, T], BF16, tag=f"xt{g}", name=f"xt{g}")
                      for g in range(NG)]

                # K and Q GEMMs (fp8 DoubleRow) + rotation (cos/sin streamed)
                for wname, w_d, brt, dst in (("k", wk8_d, bkr, krt),
                                             ("q", wq8_d, bqr, qrt)):
                    wsb = []
                    for c2 in range(NP):
                        wt = p1.tile([PD, 2, C], FP8, tag="w8", bufs=8,
                                     name=f"w{wname}{c2}")
                        nc.sync.dma_start(out=wt, in_=w_d[c2 * PD:(c2 + 1) * PD, :])
                        wsb.append(wt)
                    for e in range(NP):
                        o = e + NP
                        trig = {}
                        for ch in range(NSP):
                            csl = slice(ch * TCH, (ch + 1) * TCH)
                            cs = p1.tile([PD, TCH], BF16, tag="trig", bufs=12,
                                         name=f"cs{wname}{e}_{ch}")
                            nc.sync.dma_start(
                                out=cs, in_=cos_d[e * PD:(e + 1) * PD, csl])
                            sn = p1.tile([PD, TCH], BF16, tag="trig", bufs=12,
                                         name=f"sn{wname}{e}_{ch}")
                            nc.sync.dma_start(
                                out=sn, in_=sin_d[e * PD:(e + 1) * PD, csl])
                            trig[ch] = (cs, sn)
                        for ch in range(NSP):
                            tmp = {}
                            for g in (e, o):
                                ps = ps1.tile([PD, TCH], F32, tag="ps_mm",
                                              name=f"pkq{wname}{g}_{ch}")
                                for c2 in range(NP):
                                    nc.tensor.matmul(
                                        ps,
                                        lhsT=wsb[c2][:, :, g * PD:(g + 1) * PD],
                                        rhs=xt8[c2][:, :, ch * TCH:(ch + 1) * TCH],
                                        perf_mode=DR,
                                        start=(c2 == 0), stop=(c2 == NP - 1))
                                kt = p1.tile([PD, TCH], BF16, tag="kttmp",
                                             bufs=10, name=f"kt{wname}{g}_{ch}")
                                nc.scalar.activation(kt, ps, AF.Identity,
                                                     bias=brt[:, g:g + 1],
                                                     scale=KQ_SCALE)
                                tmp[g] = kt
                            sl = slice(ch * TCH, (ch + 1) * TCH)
                            cs, sn = trig[ch]
                            ze, zo = tmp[e], tmp[o]
                            t1 = p1.tile([PD, TCH], BF16, tag="rot", bufs=6,
                                         name=f"r1{wname}{e}_{ch}")
                            nc.vector.tensor_mul(t1, ze, cs)
                            t2 = p1.tile([PD, TCH], BF16, tag="rot", bufs=6,
                                         name=f"r2{wname}{e}_{ch}")
                            nc.vector.tensor_mul(t2, zo, sn)
                            nc.vector.tensor_add(dst[e][:, 0, sl], t1, t2)
                            t3 = p1.tile([PD, TCH], BF16, tag="rot", bufs=6,
                                         name=f"r3{wname}{e}_{ch}")
                            nc.vector.tensor_mul(t3, zo, cs)
                            t4 = p1.tile([PD, TCH], BF16, tag="rot", bufs=6,
                                         name=f"r4{wname}{e}_{ch}")
                            nc.vector.tensor_mul(t4, ze, sn)
                            nc.vector.tensor_sub(dst[e][:, 1, sl], t3, t4)

                # V GEMM last: overlaps the PE with the K/Q rotation DVE
                # chain; writes straight into resident vsb tiles so the OT
                # matmuls never wait on a DRAM round-trip
                for g in range(NG):
                    nc.sync.dma_start(out=xt[g],
                                      in_=xt_d[g * PD:(g + 1) * PD, :])
                wsb = []
                for ci in range(NG):
                    wt = p1.tile([PD, C], BF16, tag="w", bufs=8, name=f"wv{ci}")
                    nc.sync.dma_start(out=wt, in_=wv_d[ci * PD:(ci + 1) * PD, :])
                    wsb.append(wt)
                for tt in range(NT):
                    for h in range(2):
                        ps = ps1.tile([PD, TCH], F32, tag="ps_mm",
                                      name=f"pv{tt}_{h}")
                        for ci in range(NG):
                            nc.tensor.matmul(
                                ps,
                                lhsT=xt[ci][:, tt * PD:(tt + 1) * PD],
                                rhs=wsb[ci][:, h * TCH:(h + 1) * TCH],
                                start=(ci == 0), stop=(ci == NG - 1))
                        # ACT drains the psum in fp32 (no extra rounding,
                        # releases the psum ring without waiting on the
                        # rotation-busy DVE); the bias add then runs on DVE
                        # from SBUF behind the rotation chain
                        vt = p1.tile([PD, TCH], F32, tag="vtmp", bufs=4,
                                     name=f"vt{tt}_{h}")
                        nc.scalar.activation(vt, ps, AF.Copy)
                        nc.vector.tensor_add(vsb[tt][:, h * TCH:(h + 1) * TCH],
                                             vt, bvb[:, h * TCH:(h + 1) * TCH])

            # ---------------- phases 2-4: attention + projection ---------
            with tc.tile_pool(name="at", bufs=1) as at, \
                 tc.tile_pool(name="psA", bufs=4, space="PSUM") as psA, \
                 tc.tile_pool(name="psB", bufs=2, space="PSUM") as psB, \
                 tc.tile_pool(name="psC", bufs=2, space="PSUM") as psC:
                wpsb = []
                for ci in range(NG):
                    wt = at.tile([PD, C], BF16, tag=f"wp{ci}", name=f"wp{ci}")
                    nc.sync.dma_start(out=wt, in_=wp_d[ci * PD:(ci + 1) * PD, :])
                    wpsb.append(wt)

                for s in range(NSP):
                    nj = 4 * (s + 1)
                    stact = []
                    for j in range(nj):
                        # diagonal blocks (d = 0..3): columns < 128*d of the
                        # 512-wide i-span are fully below the causal mask —
                        # compute only the right 512-128*d columns
                        d = j - 4 * s
                        off = PD * d if d > 0 else 0
                        isl = slice(s * TCH + off, (s + 1) * TCH)
                        ps = psA.tile([PD, TCH], F32, tag="ps_mm",
                                      name=f"pst{s}_{j}")
                        for e in range(NP):
                            nc.tensor.matmul(
                                ps[:, off:],
                                lhsT=qrt[e][:, :, j * PD:(j + 1) * PD],
                                rhs=krt[e][:, :, isl],
                                perf_mode=DR,
                                start=(e == 0), stop=(e == NP - 1))
                        # softplus(x) = ln(1 + exp(x)); scores/sqrt(C) are
                        # bounded to a few units so exp cannot overflow
                        se = at.tile([PD, TCH], F32, tag="stexp", bufs=4,
                                     name=f"se{s}_{j}")
                        nc.scalar.activation(se[:, off:], ps[:, off:],
                                             AF.Exp, scale=SCORE_SCALE)
                        st = at.tile([PD, TCH], BF16, tag="stact", bufs=20,
                                     name=f"st{s}_{j}")
                        nc.scalar.activation(st[:, off:], se[:, off:],
                                             AF.Ln, bias=1.0)
                        if d >= 0:
                            nc.vector.tensor_mul(st[:, off:], st[:, off:],
                                                 mskt[d][:, off:])
                        stact.append((st, off))

                    ot = []
                    for g in range(NG):
                        ps2 = psB.tile([PD, TCH], F32, tag="ps_ot",
                                       name=f"pot{s}_{g}")
                        for j in range(nj):
                            st, off = stact[j]
                            nc.tensor.matmul(
                                ps2[:, off:],
                                lhsT=vsb[j][:, g * PD:(g + 1) * PD],
                                rhs=st[:, off:],
                                start=(j == 0), stop=(j == nj - 1))
                        o = at.tile([PD, TCH], BF16, tag="ot", bufs=16,
                                    name=f"ot{s}_{g}")
                        nc.scalar.activation(o, ps2, AF.Copy)
                        ot.append(o)

                    for tt in range(4):
                        trow = s * TCH + tt * PD
                        for h in range(2):
                            ps = psC.tile([PD, TCH], F32, tag="ps_pr",
                                          name=f"ppr{s}_{tt}_{h}")
                            for g in range(NG):
                                nc.tensor.matmul(
                                    ps,
                                    lhsT=ot[g][:, tt * PD:(tt + 1) * PD],
                                    rhs=wpsb[g][:, h * TCH:(h + 1) * TCH],
                                    start=(g == 0), stop=(g == NG - 1))
                            ob = at.tile([PD, TCH], F32, tag="ob", bufs=4,
                                         name=f"ob{s}_{tt}_{h}")
                            nc.vector.tensor_add(ob, ps,
                                                 bpb[:, h * TCH:(h + 1) * TCH])
                            nc.sync.dma_start(
                                out=out_d[trow:trow + PD, h * TCH:(h + 1) * TCH],
                                in_=ob)
    nc.finalize()
    return nc


def _static_tables():
    if "tables" in _CACHE:
        return _CACHE["tables"]
    perm = np.concatenate([np.arange(0, C, 2), np.arange(1, C, 2)])
    j = np.arange(H, dtype=np.float64)
    t = (T - 1 - np.arange(T)).astype(np.float64)
    ang = np.outer(j, t)                      # [H, T], angle of pair j at time t
    cosT = (np.cos(ang) * RSCALE).astype(ml_dtypes.bfloat16)
    sinT = (np.sin(ang) * RSCALE).astype(ml_dtypes.bfloat16)
    a = np.arange(PD)[:, None]
    b = np.arange(TCH)[None, :]
    masks = np.stack([(a + PD * d <= b) for d in range(NSP)])
    masks = masks.astype(ml_dtypes.bfloat16)
    _CACHE["tables"] = (perm, cosT, sinT, masks)
    return _CACHE["tables"]


def prepare(x, Wk, bk, Wq, bq, Wv, bv, Wp, bp):
    """Build (cached) the Bass program and the per-core input maps."""
    x = np.asarray(x, dtype=np.float32)
    Wk, bk = np.asarray(Wk, np.float32), np.asarray(bk, np.float32)
    Wq, bq = np.asarray(Wq, np.float32), np.asarray(bq, np.float32)
    Wv, bv = np.asarray(Wv, np.float32), np.asarray(bv, np.float32)
    Wp, bp = np.asarray(Wp, np.float32), np.asarray(bp, np.float32)

    perm, cosT, sinT, masks = _static_tables()

    def pair_fp8(arr, scale):
        """[C, F] -> [C/2, 2F] fp8: row pairs (2c2*128+p, (2c2+1)*128+p)
        interleaved along the free dim for DoubleRow contraction."""
        a = np.clip(arr * scale, -240, 240).astype(ml_dtypes.float8_e4m3fn)
        F = a.shape[1]
        a = a.reshape(NP, 2, PD, F).transpose(0, 2, 1, 3)
        return np.ascontiguousarray(a.reshape(C // 2, 2 * F))

    wk8 = pair_fp8(np.ascontiguousarray(Wk[:, perm]), W8SCALE)
    wq8 = pair_fp8(np.ascontiguousarray(Wq[:, perm]), W8SCALE)
    wv = Wv.astype(ml_dtypes.bfloat16)
    wp = Wp.astype(ml_dtypes.bfloat16)
    bkr = np.ascontiguousarray(bk[perm].reshape(NG, PD).T).astype(np.float32)
    bqr = np.ascontiguousarray(bq[perm].reshape(NG, PD).T).astype(np.float32)
    bvb = np.ascontiguousarray(np.broadcast_to(bv, (PD, C))).astype(np.float32)
    bpb = np.ascontiguousarray(np.broadcast_to(bp, (PD, C))).astype(np.float32)

    if "nc" not in _CACHE:
        _CACHE["nc"] = _build_nc()
    nc = _CACHE["nc"]

    shared = dict(wk8=wk8, wq8=wq8, wv=wv, wp=wp, bkr=bkr, bqr=bqr,
                  bvb=bvb, bpb=bpb, cosT=cosT, sinT=sinT, masks=masks)
    xb = x.astype(ml_dtypes.bfloat16)
    in_maps = []
    for i in range(NCORES):
        xti = np.ascontiguousarray(xb[i].T)
        xt8i = pair_fp8(xti.astype(np.float32), X8SCALE)
        in_maps.append(dict(xt=xti, xt8=xt8i, **shared))
    return nc, in_maps


def kernel(x, Wk, bk, Wq, bq, Wv, bv, Wp, bp):
    global LAST_RESULT
    nc, in_maps = prepare(x, Wk, bk, Wq, bq, Wv, bv, Wp, bp)
    res = run_bass_kernel_spmd(nc, in_maps, list(range(NCORES)))
    LAST_RESULT = res
    out = np.stack([res.results[i]["out"] for i in range(NCORES)], axis=0)
    return out.astype(np.float32)


# revision 29
# speedup vs baseline: 1.1999x; 1.0048x over previous
"""Trainium2 Bass kernel for nn_Attention_41996190220419.

Single-head causal attention with softplus weights and a time-flipped
rotary embedding, B=8 T=2048 C=1024 fp32.

Sharding: pure data-parallel over batch (1 batch element per NeuronCore,
8 cores, no collectives).

Per-core kernel (matmuls bf16 with fp32 PSUM accumulation, except the
score matmul which runs fp8e4 in DoubleRow mode at 2 MACs/cell/cycle):
  phase 1: xT [C,T] arrives pre-transposed from the host.
           V = x Wv + bv into resident SBUF tiles ([T,C] layout, 1024-wide
           moving operands), then KT/QT = W^T x^T (+bias via ACT), rotary
           rotation on DVE (cos/sin streamed, scaled x16) writing fp8e4
           pair-tiles [128, 2, T] (pair = channel groups e and e+4).
  phase 2: per 512-wide i-span: ST[j,i] = sum_c QR[j,c] KR[i,c] for the
           causal (block lower-triangle) blocks only via 4 DoubleRow
           matmuls, then softplus = Ln(Exp(x/(256 sqrt(C)))+1) on ACT,
           diagonal-block 0/1 masks on DVE
  phase 3: OT[c,i] = sum_j V[j,c] ST[j,i]  (PSUM accumulate over j, bf16)
  phase 4: OUT[t,cout] = sum_c OT[c,t] Wp[c,cout] + bp -> DRAM

The even/odd rotation pairs are turned into tile-level structure by
permuting the columns of Wk/Wq (and bk/bq) on the host to [evens|odds];
scores are invariant to any channel permutation applied to both K and Q.
cos/sin tables (x16, folded into the fp8 score scale) and diagonal masks
are precomputed on the host and passed as extra inputs.
"""

import os
import sys

if "/opt/trn_rl_repo" not in sys.path:
    sys.path.insert(0, "/opt/trn_rl_repo")

import numpy as np
import ml_dtypes

import concourse.bass as bass
import concourse.bacc as bacc
import concourse.mybir as mybir
import concourse.tile as tile
from concourse.bass_utils import run_bass_kernel_spmd

B, T, C = 8, 2048, 1024
H = C // 2
NCORES = 8
PD = 128
TCH = 512                 # i-span width (attention)
WCH = 1024                # moving-operand width for the dense GEMMs
NT = T // PD              # 16
NSP = T // TCH            # 4
NW = T // WCH             # 2
NG = C // PD              # 8
NP = NG // 2              # 4 fp8 pair-tiles
BF16 = mybir.dt.bfloat16
F32 = mybir.dt.float32
FP8 = mybir.dt.float8e4
AF = mybir.ActivationFunctionType
DR = mybir.MatmulPerfMode.DoubleRow
RSCALE = 16.0             # fp8 rotation output scale (folded into trig)
SCORE_SCALE = float(C) ** -0.5 / (RSCALE * RSCALE)
X8SCALE = 16.0            # fp8 x scale for the K/Q GEMMs
W8SCALE = 4096.0          # fp8 Wk/Wq scale
KQ_SCALE = 1.0 / (X8SCALE * W8SCALE)

_CACHE = {}

LAST_RESULT = None  # BassKernelResults of the most recent run (for profiling)


def _patch_act_tables():
    """Force every ACT func we use (Copy/Identity/Exp/Ln) to resolve to the
    single `natural_log_exp_and_others` table so the Exp/Ln alternation in
    the softplus does not thrash ACT_TABLE_LOADs (1.3us each).
    Table ids are positional, so keep the dict order and only strip
    functions from the other tables."""
    if _CACHE.get("act_patched"):
        return
    from concourse import hw_specs
    orig = hw_specs.get_activation_tables
    combined = "natural_log_exp_and_others"

    def patched(arch):
        tables = orig(arch)
        if combined in tables:
            keep = tables[combined]
            tables = {
                name: (s if name == combined else (s - keep))
                for name, s in tables.items()
            }
        return tables

    hw_specs.get_activation_tables = patched
    bacc.get_activation_tables = patched
    _CACHE["act_patched"] = True


def _build_nc():
    _patch_act_tables()
    nc = bacc.Bacc("TRN2", target_bir_lowering=False, debug=False,
                   num_devices=NCORES)

    xt_d = nc.dram_tensor("xt", [C, T], BF16, kind="ExternalInput").ap()
    xt8_d = nc.dram_tensor("xt8", [C // 2, 2 * T], FP8, kind="ExternalInput").ap()
    wk8_d = nc.dram_tensor("wk8", [C // 2, 2 * C], FP8, kind="ExternalInput").ap()
    wq8_d = nc.dram_tensor("wq8", [C // 2, 2 * C], FP8, kind="ExternalInput").ap()
    wv_d = nc.dram_tensor("wv", [C, C], BF16, kind="ExternalInput").ap()
    wp_d = nc.dram_tensor("wp", [C, C], BF16, kind="ExternalInput").ap()
    bkr_d = nc.dram_tensor("bkr", [PD, NG], F32, kind="ExternalInput").ap()
    bqr_d = nc.dram_tensor("bqr", [PD, NG], F32, kind="ExternalInput").ap()
    bvb_d = nc.dram_tensor("bvb", [PD, C], F32, kind="ExternalInput").ap()
    bpb_d = nc.dram_tensor("bpb", [PD, C], F32, kind="ExternalInput").ap()
    cos_d = nc.dram_tensor("cosT", [H, T], BF16, kind="ExternalInput").ap()
    sin_d = nc.dram_tensor("sinT", [H, T], BF16, kind="ExternalInput").ap()
    msk_d = nc.dram_tensor("masks", [NSP, PD, TCH], BF16,
                           kind="ExternalInput").ap()
    out_d = nc.dram_tensor("out", [T, C], F32, kind="ExternalOutput").ap()

    with tile.TileContext(nc) as tc:
        with tc.tile_pool(name="persist", bufs=1) as pp:

            # fp8 pair-tiles: dim1 index 0 -> channel group e, 1 -> e+4
            krt = [pp.tile([PD, 2, T], FP8, tag=f"krt{e}", name=f"krt{e}")
                   for e in range(NP)]
            qrt = [pp.tile([PD, 2, T], FP8, tag=f"qrt{e}", name=f"qrt{e}")
                   for e in range(NP)]
            vsb = [pp.tile([PD, C], BF16, tag=f"v{j}", name=f"v{j}")
                   for j in range(NT)]

            bkr = pp.tile([PD, NG], F32, name="bkr")
            nc.sync.dma_start(out=bkr, in_=bkr_d)
            bqr = pp.tile([PD, NG], F32, name="bqr")
            nc.sync.dma_start(out=bqr, in_=bqr_d)
            bvb = pp.tile([PD, C], F32, name="bvb")
            nc.sync.dma_start(out=bvb, in_=bvb_d)
            bpb = pp.tile([PD, C], F32, name="bpb")
            nc.sync.dma_start(out=bpb, in_=bpb_d)
            mskt = []
            for d in range(NSP):
                m = pp.tile([PD, TCH], BF16, tag=f"msk{d}", name=f"msk{d}")
                nc.sync.dma_start(out=m, in_=msk_d[d])
                mskt.append(m)

            # ---------------- phase 1: K/Q (rotated), then V --------------
            # K/Q first: they need only ~4MB of fp8 DMA before compute can
            # start, and running V afterwards hides the rotation DVE chain
            # under the V GEMMs (DVE queue order: K-rot, Q-rot, V-bias).
            with tc.tile_pool(name="p1", bufs=1) as p1, \
                 tc.tile_pool(name="ps1", bufs=8, space="PSUM") as ps1:
                xt8 = [p1.tile([PD, 2, T], FP8, tag=f"xt8_{c2}",
                               name=f"xt8_{c2}") for c2 in range(NP)]
                for c2 in range(NP):
                    nc.sync.dma_start(out=xt8[c2],
                                      in_=xt8_d[c2 * PD:(c2 + 1) * PD, :])
                xt = [p1.tile([PD, T], BF16, tag=f"xt{g}", name=f"xt{g}")
                      for g in range(NG)]

                # K and Q GEMMs (fp8 DoubleRow) + rotation (cos/sin streamed)
                for wname, w_d, brt, dst in (("k", wk8_d, bkr, krt),
                                             ("q", wq8_d, bqr, qrt)):
                    wsb = []
                    for c2 in range(NP):
                        wt = p1.tile([PD, 2, C], FP8, tag="w8", bufs=8,
                                     name=f"w{wname}{c2}")
                        nc.sync.dma_start(out=wt, in_=w_d[c2 * PD:(c2 + 1) * PD, :])
                        wsb.append(wt)
                    for e in range(NP):
                        o = e + NP
                        trig = {}
                        for ch in range(NSP):
                            csl = slice(ch * TCH, (ch + 1) * TCH)
                            cs = p1.tile([PD, TCH], BF16, tag="trig", bufs=8,
                                         name=f"cs{wname}{e}_{ch}")
                            nc.sync.dma_start(
                                out=cs, in_=cos_d[e * PD:(e + 1) * PD, csl])
                            sn = p1.tile([PD, TCH], BF16, tag="trig", bufs=8,
                                         name=f"sn{wname}{e}_{ch}")
                            nc.sync.dma_start(
                                out=sn, in_=sin_d[e * PD:(e + 1) * PD, csl])
                            trig[ch] = (cs, sn)
                        for ch in range(NSP):
                            tmp = {}
                            for g in (e, o):
                                ps = ps1.tile([PD, TCH], F32, tag="ps_mm",
                                              name=f"pkq{wname}{g}_{ch}")
                                for c2 in range(NP):
                                    nc.tensor.matmul(
                                        ps,
                                        lhsT=wsb[c2][:, :, g * PD:(g + 1) * PD],
                                        rhs=xt8[c2][:, :, ch * TCH:(ch + 1) * TCH],
                                        perf_mode=DR,
                                        start=(c2 == 0), stop=(c2 == NP - 1))
                                kt = p1.tile([PD, TCH], BF16, tag="kttmp",
                                             bufs=28, name=f"kt{wname}{g}_{ch}")
                                nc.scalar.activation(kt, ps, AF.Identity,
                                                     bias=brt[:, g:g + 1],
                                                     scale=KQ_SCALE)
                                tmp[g] = kt
                            sl = slice(ch * TCH, (ch + 1) * TCH)
                            cs, sn = trig[ch]
                            ze, zo = tmp[e], tmp[o]
                            t1 = p1.tile([PD, TCH], BF16, tag="rot", bufs=6,
                                         name=f"r1{wname}{e}_{ch}")
                            nc.vector.tensor_mul(t1, ze, cs)
                            t2 = p1.tile([PD, TCH], BF16, tag="rot", bufs=6,
                                         name=f"r2{wname}{e}_{ch}")
                            nc.vector.tensor_mul(t2, zo, sn)
                            nc.vector.tensor_add(dst[e][:, 0, sl], t1, t2)
                            t3 = p1.tile([PD, TCH], BF16, tag="rot", bufs=6,
                                         name=f"r3{wname}{e}_{ch}")
                            nc.vector.tensor_mul(t3, zo, cs)
                            t4 = p1.tile([PD, TCH], BF16, tag="rot", bufs=6,
                                         name=f"r4{wname}{e}_{ch}")
                            nc.vector.tensor_mul(t4, ze, sn)
                            nc.vector.tensor_sub(dst[e][:, 1, sl], t3, t4)

                # V GEMM last: overlaps the PE with the K/Q rotation DVE
                # chain; writes straight into resident vsb tiles so the OT
                # matmuls never wait on a DRAM round-trip
                for g in range(NG):
                    nc.sync.dma_start(out=xt[g],
                                      in_=xt_d[g * PD:(g + 1) * PD, :])
                wsb = []
                for ci in range(NG):
                    wt = p1.tile([PD, C], BF16, tag="w", bufs=8, name=f"wv{ci}")
                    nc.sync.dma_start(out=wt, in_=wv_d[ci * PD:(ci + 1) * PD, :])
                    wsb.append(wt)
                for tt in range(NT):
                    for h in range(2):
                        ps = ps1.tile([PD, TCH], F32, tag="ps_mm",
                                      name=f"pv{tt}_{h}")
                        for ci in range(NG):
                            nc.tensor.matmul(
                                ps,
                                lhsT=xt[ci][:, tt * PD:(tt + 1) * PD],
                                rhs=wsb[ci][:, h * TCH:(h + 1) * TCH],
                                start=(ci == 0), stop=(ci == NG - 1))
                        # ACT drains the psum in fp32 (no extra rounding,
                        # releases the psum ring without waiting on the
                        # rotation-busy DVE); the bias add then runs on DVE
                        # from SBUF behind the rotation chain
                        vt = p1.tile([PD, TCH], F32, tag="vtmp", bufs=4,
                                     name=f"vt{tt}_{h}")
                        nc.scalar.activation(vt, ps, AF.Copy)
                        nc.vector.tensor_add(vsb[tt][:, h * TCH:(h + 1) * TCH],
                                             vt, bvb[:, h * TCH:(h + 1) * TCH])

            # ---------------- phases 2-4: attention + projection ---------
            with tc.tile_pool(name="at", bufs=1) as at, \
                 tc.tile_pool(name="psA", bufs=4, space="PSUM") as psA, \
                 tc.tile_pool(name="psB", bufs=2, space="PSUM") as psB, \
                 tc.tile_pool(name="psC", bufs=2, space="PSUM") as psC:
                wpsb = []
                for ci in range(NG):
                    wt = at.tile([PD, C], BF16, tag=f"wp{ci}", name=f"wp{ci}")
                    nc.sync.dma_start(out=wt, in_=wp_d[ci * PD:(ci + 1) * PD, :])
                    wpsb.append(wt)

                for s in range(NSP):
                    nj = 4 * (s + 1)
                    stact = []
                    for j in range(nj):
                        # diagonal blocks (d = 0..3): columns < 128*d of the
                        # 512-wide i-span are fully below the causal mask —
                        # compute only the right 512-128*d columns
                        d = j - 4 * s
                        off = PD * d if d > 0 else 0
                        isl = slice(s * TCH + off, (s + 1) * TCH)
                        ps = psA.tile([PD, TCH], F32, tag="ps_mm",
                                      name=f"pst{s}_{j}")
                        for e in range(NP):
                            nc.tensor.matmul(
                                ps[:, off:],
                                lhsT=qrt[e][:, :, j * PD:(j + 1) * PD],
                                rhs=krt[e][:, :, isl],
                                perf_mode=DR,
                                start=(e == 0), stop=(e == NP - 1))
                        # softplus(x) = ln(1 + exp(x)); scores/sqrt(C) are
                        # bounded to a few units so exp cannot overflow
                        se = at.tile([PD, TCH], F32, tag="stexp", bufs=4,
                                     name=f"se{s}_{j}")
                        nc.scalar.activation(se[:, off:], ps[:, off:],
                                             AF.Exp, scale=SCORE_SCALE)
                        st = at.tile([PD, TCH], BF16, tag="stact", bufs=20,
                                     name=f"st{s}_{j}")
                        nc.scalar.activation(st[:, off:], se[:, off:],
                                             AF.Ln, bias=1.0)
                        if d >= 0:
                            nc.vector.tensor_mul(st[:, off:], st[:, off:],
                                                 mskt[d][:, off:])
                        stact.append((st, off))

                    ot = []
                    for g in range(NG):
                        ps2 = psB.tile([PD, TCH], F32, tag="ps_ot",
                                       name=f"pot{s}_{g}")
                        for j in range(nj):
                            st, off = stact[j]
                            nc.tensor.matmul(
                                ps2[:, off:],
                                lhsT=vsb[j][:, g * PD:(g + 1) * PD],
                                rhs=st[:, off:],
                                start=(j == 0), stop=(j == nj - 1))
                        o = at.tile([PD, TCH], BF16, tag="ot", bufs=16,
                                    name=f"ot{s}_{g}")
                        nc.scalar.activation(o, ps2, AF.Copy)
                        ot.append(o)

                    for tt in range(4):
                        trow = s * TCH + tt * PD
                        for h in range(2):
                            ps = psC.tile([PD, TCH], F32, tag="ps_pr",
                                          name=f"ppr{s}_{tt}_{h}")
                            for g in range(NG):
                                nc.tensor.matmul(
                                    ps,
                                    lhsT=ot[g][:, tt * PD:(tt + 1) * PD],
                                    rhs=wpsb[g][:, h * TCH:(h + 1) * TCH],
                                    start=(g == 0), stop=(g == NG - 1))
                            ob = at.tile([PD, TCH], F32, tag="ob", bufs=4,
                                         name=f"ob{s}_{tt}_{h}")
                            nc.vector.tensor_add(ob, ps,
                                                 bpb[:, h * TCH:(h + 1) * TCH])
                            nc.sync.dma_start(
                                out=out_d[trow:trow + PD, h * TCH:(h + 1) * TCH],
                                in_=ob)
    nc.finalize()
    return nc


def _static_tables():
    if "tables" in _CACHE:
        return _CACHE["tables"]
    perm = np.concatenate([np.arange(0, C, 2), np.arange(1, C, 2)])
    j = np.arange(H, dtype=np.float64)
    t = (T - 1 - np.arange(T)).astype(np.float64)
    ang = np.outer(j, t)                      # [H, T], angle of pair j at time t
    cosT = (np.cos(ang) * RSCALE).astype(ml_dtypes.bfloat16)
    sinT = (np.sin(ang) * RSCALE).astype(ml_dtypes.bfloat16)
    a = np.arange(PD)[:, None]
    b = np.arange(TCH)[None, :]
    masks = np.stack([(a + PD * d <= b) for d in range(NSP)])
    masks = masks.astype(ml_dtypes.bfloat16)
    _CACHE["tables"] = (perm, cosT, sinT, masks)
    return _CACHE["tables"]


def prepare(x, Wk, bk, Wq, bq, Wv, bv, Wp, bp):
    """Build (cached) the Bass program and the per-core input maps."""
    x = np.asarray(x, dtype=np.float32)
    Wk, bk = np.asarray(Wk, np.float32), np.asarray(bk, np.float32)
    Wq, bq = np.asarray(Wq, np.float32), np.asarray(bq, np.float32)
    Wv, bv = np.asarray(Wv, np.float32), np.asarray(bv, np.float32)
    Wp, bp = np.asarray(Wp, np.float32), np.asarray(bp, np.float32)

    perm, cosT, sinT, masks = _static_tables()

    def pair_fp8(arr, scale):
        """[C, F] -> [C/2, 2F] fp8: row pairs (2c2*128+p, (2c2+1)*128+p)
        interleaved along the free dim for DoubleRow contraction."""
        a = np.clip(arr * scale, -240, 240).astype(ml_dtypes.float8_e4m3fn)
        F = a.shape[1]
        a = a.reshape(NP, 2, PD, F).transpose(0, 2, 1, 3)
        return np.ascontiguousarray(a.reshape(C // 2, 2 * F))

    wk8 = pair_fp8(np.ascontiguousarray(Wk[:, perm]), W8SCALE)
    wq8 = pair_fp8(np.ascontiguousarray(Wq[:, perm]), W8SCALE)
    wv = Wv.astype(ml_dtypes.bfloat16)
    wp = Wp.astype(ml_dtypes.bfloat16)
    bkr = np.ascontiguousarray(bk[perm].reshape(NG, PD).T).astype(np.float32)
    bqr = np.ascontiguousarray(bq[perm].reshape(NG, PD).T).astype(np.float32)
    bvb = np.ascontiguousarray(np.broadcast_to(bv, (PD, C))).astype(np.float32)
    bpb = np.ascontiguousarray(np.broadcast_to(bp, (PD, C))).astype(np.float32)

    if "nc" not in _CACHE:
        _CACHE["nc"] = _build_nc()
    nc = _CACHE["nc"]

    shared = dict(wk8=wk8, wq8=wq8, wv=wv, wp=wp, bkr=bkr, bqr=bqr,
                  bvb=bvb, bpb=bpb, cosT=cosT, sinT=sinT, masks=masks)
    xb = x.astype(ml_dtypes.bfloat16)
    in_maps = []
    for i in range(NCORES):
        xti = np.ascontiguousarray(xb[i].T)
        xt8i = pair_fp8(xti.astype(np.float32), X8SCALE)
        in_maps.append(dict(xt=xti, xt8=xt8i, **shared))
    return nc, in_maps


def kernel(x, Wk, bk, Wq, bq, Wv, bv, Wp, bp):
    global LAST_RESULT
    nc, in_maps = prepare(x, Wk, bk, Wq, bq, Wv, bv, Wp, bp)
    res = run_bass_kernel_spmd(nc, in_maps, list(range(NCORES)))
    LAST_RESULT = res
    out = np.stack([res.results[i]["out"] for i in range(NCORES)], axis=0)
    return out.astype(np.float32)
